# revision 73
# baseline (speedup 1.0000x reference)
"""Additive attention (B=4, Q=512, K=512, D=256, H=256) on 8 TRN2 NeuronCores.

Sharding: data-parallel over query rows. Core c owns q-rows [c*64, (c+1)*64)
of every batch; every core holds all keys/values/weights. No collectives.

Per-core pipeline (per batch b, JIT-specialized to valid_lens[b]):
  PE : qf = Wq^T q, kf = Wk^T k              (bf16 matmuls, H on partitions)
  DVE: pre[h,k] = kf[h,k] + qf[h,q]          (tensor_scalar, per-partition scalar)
  ACT: feat = tanh(pre)                      (batched 16 q's per instruction)
  PE : scoresT[k,q] = (feat as weights)^T wv (k on PSUM partitions, q on free)
  ACT: expT[k,q] = exp(scoresT)              (masked tail rows pre-set to -40)
  PE : out[q,:] = expT^T @ [values | 1]      (ones col gives the softmax denom)
  DVE: out[:, :D] *= 1/out[:, D]
"""

from contextlib import ExitStack

import ml_dtypes
import numpy as np

import concourse.bass as bass
import concourse.mybir as mybir
import concourse.tile as tile
from concourse import bacc
from concourse.bass_utils import run_bass_kernel_spmd

B, Q, K, D, H = 4, 512, 512, 256, 256
DA = D + 1  # values with an appended ones column
NCORES = 8
QL = Q // NCORES  # 64 q-rows per core
G = 16  # q's per tanh batch
BF16 = mybir.dt.bfloat16
F16 = mybir.dt.float16
F32 = mybir.dt.float32
PRE_DT = BF16  # dtype of kf/qf/pre (bf16 -> DVE 4x preadds, halved SBUF)
AF = mybir.ActivationFunctionType
OP = mybir.AluOpType

# DVE-side tanh: the ACT engine is the bottleneck (1 elem/cycle/lane), so a
# few q's per batch compute tanh on the otherwise-idle VectorE instead, as a
# clamped degree-5 odd polynomial in fp16 (|err| < 7e-3):
#   tanh(x) ~ xc*P(xc^2), xc = clamp(x, +-XC)
ND = 4  # q's per batch on the DVE path
XC = 3.8
TANH_C = [0.9722264069539809, -0.24794917779739903, 0.047820199641127936,
          -0.0052199270152815365, 0.0002882985131364362,
          -6.243137462422935e-06]

_build_cache: dict = {}
last_results = None  # BassKernelResults of the most recent kernel() call


def _k_use(v: int) -> int:
    # columns actually computed for a batch: valid len rounded up to a
    # multiple of 4 (even FD keeps DVE tensor_scalar in 2x mode)
    return min(K, max(4, ((int(v) + 3) // 4) * 4))


def build(valid_lens, repeat: int = 1) -> bacc.Bacc:
    valid = tuple(int(v) for v in valid_lens)
    ku = [_k_use(v) for v in valid]

    nc = bacc.Bacc("TRN2", target_bir_lowering=False, debug=False,
                   enable_asserts=False)

    # all per-core tensors are laid out partition-major with the whole
    # per-partition payload contiguous (1-2KB DMA lines, one DMA each)
    qT = nc.dram_tensor("qT", [128, 2, B * QL], BF16, kind="ExternalInput").ap()
    kT = nc.dram_tensor("kT", [B, 128, 2, K], BF16, kind="ExternalInput").ap()
    vals = nc.dram_tensor("vals", [B, 128, 4, DA], BF16, kind="ExternalInput").ap()
    wq = nc.dram_tensor("wq", [128, 2, H], BF16, kind="ExternalInput").ap()
    wk = nc.dram_tensor("wk", [128, 2, H], BF16, kind="ExternalInput").ap()
    wv2 = nc.dram_tensor("wv2", [128, 2], BF16, kind="ExternalInput").ap()
    wv16 = nc.dram_tensor("wv16", [128, 2], F16, kind="ExternalInput").ap()
    # per-batch mask bias column for the last k-tile's exp: 0 on valid rows,
    # -40 on the rounded-up tail (exp(-40) ~ 4e-18 ~ 0)
    mb = nc.dram_tensor("mb", [128, B], F32, kind="ExternalInput").ap()
    out = nc.dram_tensor("out", [B, QL, D], F32, kind="ExternalOutput").ap()

    with tile.TileContext(nc) as tc, ExitStack() as ctx:
        cp = ctx.enter_context(tc.tile_pool(name="consts", bufs=1))
        sb = ctx.enter_context(tc.tile_pool(name="sbuf", bufs=2))
        exp_pool = ctx.enter_context(tc.tile_pool(name="expp", bufs=5))
        small = ctx.enter_context(tc.tile_pool(name="small", bufs=4))
        pre_pool = ctx.enter_context(tc.tile_pool(name="pre", bufs=2))
        feat_pool = ctx.enter_context(tc.tile_pool(name="feat", bufs=2))
        dvp = ctx.enter_context(tc.tile_pool(name="dvp", bufs=1))
        xcp = ctx.enter_context(tc.tile_pool(name="xcp", bufs=2))
        ps_proj = ctx.enter_context(tc.tile_pool(name="ps_proj", bufs=4, space="PSUM"))
        ps_sc = ctx.enter_context(tc.tile_pool(name="ps_sc", bufs=2, space="PSUM"))
        ps_out = ctx.enter_context(tc.tile_pool(name="ps_out", bufs=2, space="PSUM"))

        # prime the ACT table load (tanh/exp share one set) at t~0 so the
        # ~2.7us load is off the critical path
        primer = cp.tile([1, 1], F32, tag="primer")
        nc.gpsimd.memset(primer[:, :], 0.0)
        nc.scalar.activation(primer[:, :], primer[:, :], AF.Tanh)

        wq_sb = cp.tile([128, 2, H], BF16, tag="wq")
        wk_sb = cp.tile([128, 2, H], BF16, tag="wk")
        wv_sb = cp.tile([128, 2], BF16, tag="wv")
        wv16_sb = cp.tile([128, 2], F16, tag="wv16")
        qT_sb = cp.tile([128, 2, B * QL], BF16, tag="qT")
        mb_sb = cp.tile([128, B], F32, tag="mb")

        # process batches small-to-large so the serial prologue (DMA ->
        # projection -> preadd -> first tanh) is as short as possible
        border = sorted(range(B), key=lambda b: ku[b])
        seq = [b for _ in range(repeat) for b in border]
        b0 = seq[0]

        # spread the prologue-critical DMAs over different engines' DMA
        # queues so they transfer in parallel
        kT_tiles = {}
        kT_tiles[b0] = sb.tile([128, 2, K], BF16, tag="kT", name=f"kT_{b0}")
        nc.scalar.dma_start(kT_tiles[b0][:, :, :], kT[b0])
        nc.sync.dma_start(wk_sb[:, :, :], wk[:, :, :])
        nc.sync.dma_start(wq_sb[:, :, :], wq[:, :, :])
        nc.gpsimd.dma_start(qT_sb[:, :, :], qT[:, :, :])
        nc.sync.dma_start(wv_sb[:, :], wv2[:, :])
        nc.sync.dma_start(wv16_sb[:, :], wv16[:, :])
        nc.sync.dma_start(mb_sb[:, :], mb[:, :])

        kf_tiles = {}

        def project_kf(b, rhs_fn, uniq):
            Ku_ = ku[b]
            kf_t = sb.tile([128, 2, K], PRE_DT, tag="kf", name=f"kf_{uniq}")
            for h2 in range(2):
                ps = ps_proj.tile([128, K], F32, tag="proj",
                                  name=f"kfp_{uniq}_{h2}")
                for dt in range(2):
                    nc.tensor.matmul(
                        ps[:, :Ku_],
                        lhsT=wk_sb[:, dt, bass.ts(h2, 128)],
                        rhs=rhs_fn(dt, Ku_),
                        start=(dt == 0), stop=(dt == 1),
                    )
                nc.vector.tensor_copy(kf_t[:, h2, :Ku_], ps[:, :Ku_])
            return kf_t

        # first batch's kf projection heads the critical path: emit it
        # before the qf projection
        kf_tiles[b0] = project_kf(
            b0, lambda dt, ku_: kT_tiles[b0][:, dt, :ku_], "p0")

        # qf[h, (b,q)] for all batches, H split in two 128-halves
        # (f32: the tensor_scalar per-partition operand must be float32;
        #  copies go on the otherwise-idle ACT engine)
        qf_sb = cp.tile([128, 2, B * QL], F32, tag="qf")
        for h2 in range(2):
            ps = ps_proj.tile([128, B * QL], F32, tag="proj",
                              name=f"qfp_{h2}")
            for dt in range(2):
                nc.tensor.matmul(
                    ps[:, :],
                    lhsT=wq_sb[:, dt, bass.ts(h2, 128)],
                    rhs=qT_sb[:, dt, :],
                    start=(dt == 0), stop=(dt == 1),
                )
            nc.scalar.copy(qf_sb[:, h2, :], ps[:, :])

        for bi, b in enumerate(seq):
            Ku = ku[b]
            nkt = (Ku + 127) // 128
            kT_sb = kT_tiles.pop(b, None)  # b0's kf was projected up front

            # prefetch next batch's keys while this batch computes
            if bi + 1 < len(seq):
                nb = seq[bi + 1]
                kT_tiles[nb] = sb.tile([128, 2, K], BF16, tag="kT",
                                       name=f"kT_{bi + 1}_{nb}")
                nc.sync.dma_start(kT_tiles[nb][:, :, :], kT[nb])

            # scoresT[k, q] accumulated in one PSUM bank: [128, (kt, q)]
            sc_ps = ps_sc.tile([128, nkt, QL], F32, tag="sc")

            # kf[h, k] for this batch (first batch was projected up front)
            kf_sb = kf_tiles.pop(b, None)
            if kf_sb is None:
                kf_sb = project_kf(
                    b, lambda dt, ku_: kT_sb[:, dt, :ku_], f"i{bi}")

            # first groups of the first batch are small so ACT ramps up fast;
            # the last ND q's go to the DVE-polynomial path instead of ACT
            # (only for batches big enough that ACT is the bottleneck)
            nd_b = ND if Ku >= 256 else 0
            nm = QL - nd_b
            groups = [2, 6, 8, 16, 16, nm - 48] if bi == 0 else \
                     [16, 16, 16, 8, nm - 58, 2] if bi == len(seq) - 1 else \
                     [G] * (nm // G) + ([nm % G] if nm % G else [])
            # one batched DVE chain per batch, emitted after the main groups
            # (measured best: its DVE burst drains while ACT chews the last
            # queued tanh groups and the next batch's projections run)
            def emit_dve_group(uniq):
                pred = dvp.tile([128, nd_b, 2, Ku], PRE_DT, tag="pred",
                                name=f"pred_{uniq}")
                for j in range(nd_b):
                    col = b * QL + nm + j
                    for h2 in range(2):
                        nc.vector.tensor_scalar_add(
                            pred[:, j, h2, :],
                            kf_sb[:, h2, :Ku],
                            qf_sb[:, h2, col:col + 1],
                        )
                xc = xcp.tile([128, nd_b, 2, Ku], F16, tag="xc",
                              name=f"xc_{uniq}")
                nc.vector.tensor_scalar(xc[:, :, :, :], pred[:, :, :, :],
                                        XC, -XC, op0=OP.min, op1=OP.max)
                td = dvp.tile([128, nd_b, 2, Ku], F16, tag="td",
                              name=f"td_{uniq}")
                nc.vector.tensor_mul(td[:, :, :, :], xc[:, :, :, :],
                                     xc[:, :, :, :])
                ud = dvp.tile([128, nd_b, 2, Ku], F16, tag="ud",
                              name=f"ud_{uniq}")
                nc.vector.tensor_scalar(ud[:, :, :, :], td[:, :, :, :],
                                        TANH_C[5], TANH_C[4],
                                        op0=OP.mult, op1=OP.add)
                for ck in (3, 2, 1, 0):
                    nc.vector.tensor_mul(ud[:, :, :, :], ud[:, :, :, :],
                                         td[:, :, :, :])
                    nc.vector.tensor_scalar_add(ud[:, :, :, :],
                                                ud[:, :, :, :], TANH_C[ck])
                nc.vector.tensor_mul(xc[:, :, :, :], ud[:, :, :, :],
                                     xc[:, :, :, :])
                for kt in range(nkt):
                    cs = min(128, Ku - kt * 128)
                    for j in range(nd_b):
                        qq = nm + j
                        for h2 in range(2):
                            nc.tensor.matmul(
                                sc_ps[:cs, kt, qq:qq + 1],
                                lhsT=xc[:, j, h2, kt * 128:kt * 128 + cs],
                                rhs=wv16_sb[:, h2:h2 + 1],
                                start=(h2 == 0), stop=(h2 == 1),
                            )

            q0 = 0
            for g, gs in enumerate(groups):
                pre = pre_pool.tile([128, gs, 2, Ku], PRE_DT, tag="pre")
                feat = feat_pool.tile([128, gs, 2, Ku], BF16, tag="feat")
                for j in range(gs):
                    col = b * QL + q0 + j
                    for h2 in range(2):
                        nc.vector.tensor_scalar_add(
                            pre[:, j, h2, :],
                            kf_sb[:, h2, :Ku],
                            qf_sb[:, h2, col:col + 1],
                        )
                nc.scalar.activation(feat[:, :, :, :], pre[:, :, :, :], AF.Tanh)
                # kt-outer: the last group completes k-tile 0's scores first,
                # so exp/output-matmul overlap the remaining matvecs
                for kt in range(nkt):
                    cs = min(128, Ku - kt * 128)
                    for j in range(gs):
                        qq = q0 + j
                        for h2 in range(2):
                            nc.tensor.matmul(
                                sc_ps[:cs, kt, qq:qq + 1],
                                lhsT=feat[:, j, h2, kt * 128:kt * 128 + cs],
                                rhs=wv_sb[:, h2:h2 + 1],
                                start=(h2 == 0), stop=(h2 == 1),
                            )
                q0 += gs
                if nd_b and g == len(groups) - 2:
                    emit_dve_group(f"{bi}")

            # values are only needed for the epilogue matmul; DMA them late
            vals_sb = sb.tile([128, 4, DA], BF16, tag="vals")
            nc.sync.dma_start(vals_sb[:, :nkt, :], vals[b, :, :nkt, :])

            # exp (mask = per-partition bias on the last k-tile), attn @ values
            cs_l = Ku - (nkt - 1) * 128
            masked = valid[b] < Ku
            e_all = exp_pool.tile([128, nkt, QL], BF16, tag="exp")
            if nkt > 1:
                nc.scalar.activation(e_all[:, :nkt - 1, :],
                                     sc_ps[:, :nkt - 1, :], AF.Exp)
            nc.scalar.activation(
                e_all[:cs_l, nkt - 1, :], sc_ps[:cs_l, nkt - 1, :], AF.Exp,
                bias=mb_sb[:cs_l, b:b + 1] if masked else 0.0,
            )
            out_ps = ps_out.tile([QL, DA], F32, tag="out")
            for kt in range(nkt):
                cs = min(128, Ku - kt * 128)
                nc.tensor.matmul(
                    out_ps[:, :],
                    lhsT=e_all[:cs, kt, :],
                    rhs=vals_sb[:cs, kt, :],
                    start=(kt == 0), stop=(kt == nkt - 1),
                )

            rcp = small.tile([QL, 1], F32, tag="rcp")
            nc.vector.reciprocal(rcp[:, :], out_ps[:, D:DA])
            out_sb = sb.tile([QL, D], F32, tag="osb")
            nc.vector.tensor_scalar_mul(out_sb[:, :], out_ps[:, :D], rcp[:, 0:1])
            nc.sync.dma_start(out[b], out_sb[:, :])

    nc.compile()
    return nc


def prep_inputs(queries, keys, values, Wq, Wk, wv, valid_lens):
    """Host-side layout prep (transposes/casts only). Returns per-core in_maps."""
    bf = ml_dtypes.bfloat16
    valid = [int(v) for v in valid_lens]
    kT = np.ascontiguousarray(
        keys.transpose(0, 2, 1).reshape(B, 2, 128, K).transpose(0, 2, 1, 3)
    ).astype(bf)  # [B, 128, 2, K]
    va = np.ones((B, K, DA), dtype=np.float32)
    va[:, :, :D] = values
    vals = np.ascontiguousarray(
        va.reshape(B, 4, 128, DA).transpose(0, 2, 1, 3)
    ).astype(bf)  # [B, 128, 4, DA]
    wq = np.ascontiguousarray(
        Wq.reshape(2, 128, H).transpose(1, 0, 2)).astype(bf)  # [128, 2, H]
    wk = np.ascontiguousarray(
        Wk.reshape(2, 128, H).transpose(1, 0, 2)).astype(bf)
    wv2 = np.ascontiguousarray(wv.reshape(2, 128).T).astype(bf)
    wv16 = np.ascontiguousarray(wv.reshape(2, 128).T).astype(np.float16)
    mb = np.zeros((128, B), dtype=np.float32)
    for b in range(B):
        lastk0 = ((_k_use(valid[b]) + 127) // 128 - 1) * 128
        mb[:, b] = np.where(lastk0 + np.arange(128) < valid[b], 0.0, -40.0)
    in_maps = []
    for c in range(NCORES):
        qs = queries[:, c * QL:(c + 1) * QL, :]  # [B, QL, D]
        qTc = np.ascontiguousarray(
            qs.transpose(2, 0, 1).reshape(2, 128, B * QL).transpose(1, 0, 2)
        ).astype(bf)  # [128, 2, B*QL]
        in_maps.append({
            "qT": qTc, "kT": kT, "vals": vals,
            "wq": wq, "wk": wk, "wv2": wv2, "wv16": wv16, "mb": mb,
        })
    return in_maps


def kernel(queries, keys, values, Wq, Wk, wv, valid_lens) -> np.ndarray:
    global last_results
    queries = np.asarray(queries, dtype=np.float32)
    keys = np.asarray(keys, dtype=np.float32)
    values = np.asarray(values, dtype=np.float32)
    Wq = np.asarray(Wq, dtype=np.float32)
    Wk = np.asarray(Wk, dtype=np.float32)
    wv = np.asarray(wv, dtype=np.float32)
    valid = tuple(int(v) for v in np.asarray(valid_lens))

    if valid not in _build_cache:
        _build_cache[valid] = build(valid)
    nc = _build_cache[valid]

    in_maps = prep_inputs(queries, keys, values, Wq, Wk, wv, valid)
    try:
        res = run_bass_kernel_spmd(nc, in_maps, core_ids=list(range(NCORES)))
    except Exception:
        # transient NRT device errors (wedged core) usually clear on retry
        res = run_bass_kernel_spmd(nc, in_maps, core_ids=list(range(NCORES)))
    last_results = res

    full = np.empty((B, Q, D), dtype=np.float32)
    for c in range(NCORES):
        oc = res.results[c]["out"]  # [B, QL, D]
        for b in range(B):
            full[b, c * QL:(c + 1) * QL, :] = oc[b]
    return full

